# revision 10
# baseline (speedup 1.0000x reference)
"""Trainium2 Bass kernel for nn_LinearUpscaler (masked embedding-bag sum + bias).

reference:  g = W.T[ids]; g[ids == 0] = 0; out = g.sum(axis=2) + b

Design: data-parallel over batch across 8 cores (8 batch rows -> 1600 bags of
51 slots each: 50 items + 1 bias-row slot; ids==0 remapped to a zero row).

The gather engine is the GPSIMD dma_gather custom op (int16 indices, one
512B/256B row per index, written to partition i%128, column i//128).  Since
indices are signed int16 (max 32767) and V=100002, the fp16 table is split
into 4 vocab chunks; row 0 of each chunk is a zero row so padding slots can
gather harmlessly.  For each tile of 128 bags, each bag's slots are bucketed
by chunk; per (tile, chunk) all bags are padded to a common width W so the
dma_gather index list is fully valid (no negative indices) with a
compile-time num_idxs.  The list order is chosen so bag b's rows land in
partition b at consecutive columns; one strided vector-engine reduce per tile
sums items+chunks+padding (pads contribute zeros) in a single pass, and the
result is DMA'd out.  The program is specialized per call (widths depend on
the actual ids); no collectives are needed.
"""

import os
import sys

for _p in ("/opt/trn_rl_repo", "/root/.axon_site/_ro/trn_rl_repo"):
    if os.path.isdir(_p) and _p not in sys.path:
        sys.path.insert(0, _p)
        break

import numpy as np

N_CORES = 8
B, S, K = 64, 200, 50
V, E = 100000, 128
KE = K + 1            # items + bias slot
BIAS_V = V            # logical row V   = b
ZERO_V = V + 1        # logical row V+1 = 0
NV = V + 2            # logical vocab incl. bias+zero rows
P = 128
B_PER_CORE = B // N_CORES
BAGS = B_PER_CORE * S               # 1600 bags per core
N_TILES = -(-BAGS // P)             # 13
BAGS_PAD = N_TILES * P              # 1664

CHUNK_CAP = 32767                   # real rows per chunk (idx 1..32767)
N_CHUNKS = -(-NV // CHUNK_CAP)      # 4
CHUNK_STRIDE = 32768                # rows per chunk incl. its zero row
CHUNK_ROWS = [min(CHUNK_CAP, NV - CHUNK_CAP * c) + 1 for c in range(N_CHUNKS)]

TRACE = False       # test.py flips this to profile
LAST_RESULTS = {}   # test.py reads exec_time_ns etc. from here


def _build_tables(W, b):
    """fp16 chunk tables, each [zero row; <=CHUNK_CAP vocab rows].

    Separate tensors (not slices of one): the gather ucode's row addressing
    breaks when AP-base-offset + idx exceeds 32767 rows."""
    wt = np.zeros((NV, E), np.float32)
    wt[:V] = W.T
    wt[BIAS_V] = b
    tabs = []
    for c in range(N_CHUNKS):
        real = CHUNK_ROWS[c] - 1
        t = np.zeros((CHUNK_ROWS[c], E), np.float16)
        t[1:] = wt[CHUNK_CAP * c : CHUNK_CAP * c + real]
        tabs.append(t)
    return tabs


def _plan_core(v_bags):
    """v_bags: [BAGS, KE] logical rows. Returns per-(tile,chunk) raw widths and
    per-bag chunk-sorted idx lists.

    sorted_idx[bag, j] = local int16 idx of the bag's j-th slot when slots are
    ordered by chunk; cnt[bag, c] = number of slots in chunk c."""
    C = v_bags // CHUNK_CAP                      # [BAGS, KE] chunk of each slot
    IDX = (v_bags - C * CHUNK_CAP + 1).astype(np.int16)
    order = np.argsort(C, axis=1, kind="stable")  # chunk-major slot order
    C_sorted = np.take_along_axis(C, order, axis=1)
    IDX_sorted = np.take_along_axis(IDX, order, axis=1)
    cnt = np.stack([(C == c).sum(axis=1) for c in range(N_CHUNKS)], axis=1)
    return C_sorted, IDX_sorted, cnt


def _wrap_idxs(arr, w):
    """arr [P, w] int16 (partition-major slot grid) -> [128, w*8] wrapped+replicated."""
    L = P * w
    i = np.arange(L)
    lin = arr[i % P, i // P]                     # list position i = col*128 + p
    wrapped = lin.reshape(w * 8, 16).T           # [16, w*8]
    return np.tile(wrapped, (8, 1)).astype(np.int16)


def _prep_inputs(content_input, W, b):
    """Returns (in_maps, widths) where widths[t][c] is shared across cores."""
    ids = np.asarray(content_input).astype(np.int64).reshape(B, S, K)
    Wf = np.asarray(W, dtype=np.float32)
    bf = np.asarray(b, dtype=np.float32)
    tabs = _build_tables(Wf, bf)

    ids = np.where(ids == 0, ZERO_V, ids)
    per_core = []
    for c in range(N_CORES):
        bag_ids = ids[c * B_PER_CORE : (c + 1) * B_PER_CORE].reshape(BAGS, K)
        v = np.concatenate(
            [bag_ids, np.full((BAGS, 1), BIAS_V, np.int64)], axis=1
        )  # [BAGS, KE]
        per_core.append(_plan_core(v))

    # raw widths per (core, tile, chunk); dummy bags (tile 12 rows 64:128) have
    # no slots at all
    widths = np.zeros((N_TILES, N_CHUNKS), np.int64)
    for c in range(N_CORES):
        _, _, cnt = per_core[c]
        for t in range(N_TILES):
            rows = np.arange(t * P, min((t + 1) * P, BAGS))
            widths[t] = np.maximum(widths[t], cnt[rows].max(axis=0))

    in_maps = []
    for core in range(N_CORES):
        C_sorted, IDX_sorted, cnt = per_core[core]
        planes = []
        for t in range(N_TILES):
            lo, hi = t * P, min((t + 1) * P, BAGS)
            nrow = hi - lo
            csum = np.zeros((P,), np.int64)
            for ch in range(N_CHUNKS):
                w = int(widths[t][ch])
                if w == 0:
                    continue
                arr = np.zeros((P, w), np.int16)
                if nrow:
                    cn = cnt[lo:hi, ch]
                    j = np.arange(w)[None, :]
                    src = np.take_along_axis(
                        IDX_sorted[lo:hi],
                        np.minimum(csum[:nrow, None] + j, KE - 1),
                        axis=1,
                    )
                    arr[:nrow] = np.where(j < cn[:, None], src, 0)
                planes.append(_wrap_idxs(arr, w))
                csum[:nrow] += cnt[lo:hi, ch]
        ids16 = np.concatenate(planes, axis=1)
        m = {"ids16": np.ascontiguousarray(ids16)}
        for c in range(N_CHUNKS):
            m[f"wt{c}"] = tabs[c]
        in_maps.append(m)
    return in_maps, widths


def _build_program(widths, ids_cols):
    import concourse.bass as bass
    import concourse.mybir as mybir
    from concourse import bacc
    from concourse.tile import TileContext

    sw = [int(widths[t].sum()) for t in range(N_TILES)]
    sw_max = max(sw)

    nc = bacc.Bacc("TRN2", target_bir_lowering=False, debug=False, num_devices=N_CORES)
    ids_d = nc.declare_dram_parameter("ids16", [P, ids_cols], mybir.dt.int16, isOutput=False)
    wt_ds = [
        nc.declare_dram_parameter(
            f"wt{c}", [CHUNK_ROWS[c], E], mybir.dt.float16, isOutput=False
        )
        for c in range(N_CHUNKS)
    ]
    out_d = nc.declare_dram_parameter("out", [BAGS_PAD, E], mybir.dt.float32, isOutput=True)

    with TileContext(nc) as tc:
        with (
            tc.tile_pool(name="ids", bufs=1) as ids_pool,
            tc.tile_pool(name="g", bufs=3) as g_pool,
            tc.tile_pool(name="o", bufs=3) as o_pool,
        ):
            ids_sb = ids_pool.tile([P, ids_cols], mybir.dt.int16)
            nc.sync.dma_start(out=ids_sb[:], in_=ids_d[:])
            off = 0  # free-dim offset into ids16, in idx elements
            for t in range(N_TILES):
                g = g_pool.tile([P, sw_max * E], mybir.dt.float16)
                col = 0
                for ch in range(N_CHUNKS):
                    w = int(widths[t][ch])
                    if w == 0:
                        continue
                    n = w * P
                    dst = g[:, col * E : (col + w) * E].rearrange(
                        "p (j e) -> p j e", j=w, e=E
                    )
                    nc.gpsimd.dma_gather(
                        dst,
                        wt_ds[ch][:],
                        ids_sb[:, off : off + w * 8],
                        n,
                        n,
                        E,
                        single_packet=False,
                    )
                    col += w
                    off += w * 8
                o = o_pool.tile([P, E], mybir.dt.float32)
                nc.vector.tensor_reduce(
                    out=o[:],
                    in_=g[:, : sw[t] * E].rearrange("p (j e) -> p e j", j=sw[t], e=E),
                    axis=mybir.AxisListType.X,
                    op=mybir.AluOpType.add,
                )
                nc.sync.dma_start(out=out_d[t * P : (t + 1) * P, :], in_=o[:])
    nc.compile()
    return nc


def kernel(content_input, W, b):
    global LAST_RESULTS
    in_maps, widths = _prep_inputs(content_input, W, b)
    ids_cols = in_maps[0]["ids16"].shape[1]
    nc = _build_program(widths, ids_cols)

    from concourse.bass_utils import run_bass_kernel_spmd

    res = run_bass_kernel_spmd(nc, in_maps, list(range(N_CORES)), trace=TRACE)
    LAST_RESULTS = {
        "exec_time_ns": res.exec_time_ns,
        "mean_exec_time_ns": res.mean_exec_time_ns,
        "instructions_and_trace": res.instructions_and_trace,
        "profile_json": res.profile_json,
        "widths": widths,
    }

    out = np.empty((B, S, E), np.float32)
    for c in range(N_CORES):
        out[c * B_PER_CORE : (c + 1) * B_PER_CORE] = (
            res.results[c]["out"][:BAGS].reshape(B_PER_CORE, S, E)
        )
    return out


# revision 13
# speedup vs baseline: 1.3280x; 1.3280x over previous
"""Trainium2 Bass kernel for nn_LinearUpscaler (masked embedding-bag sum + bias).

reference:  g = W.T[ids]; g[ids == 0] = 0; out = g.sum(axis=2) + b

Design: data-parallel over batch across 8 cores (8 batch rows -> 1600 bags of
51 slots each: 50 items + 1 bias-row slot; ids==0 remapped to a zero row).

The gather engine is the GPSIMD dma_gather custom op (int16 indices, one
512B/256B row per index, written to partition i%128, column i//128).  Since
indices are signed int16 (max 32767) and V=100002, the fp16 table is split
into 4 vocab chunks; row 0 of each chunk is a zero row so padding slots can
gather harmlessly.  For each tile of 128 bags, each bag's slots are bucketed
by chunk; per (tile, chunk) all bags are padded to a common width W so the
dma_gather index list is fully valid (no negative indices) with a
compile-time num_idxs.  The list order is chosen so bag b's rows land in
partition b at consecutive columns; one strided vector-engine reduce per tile
sums items+chunks+padding (pads contribute zeros) in a single pass, and the
result is DMA'd out.  The program is specialized per call (widths depend on
the actual ids); no collectives are needed.
"""

import os
import sys

for _p in ("/opt/trn_rl_repo", "/root/.axon_site/_ro/trn_rl_repo"):
    if os.path.isdir(_p) and _p not in sys.path:
        sys.path.insert(0, _p)
        break

import numpy as np

N_CORES = 8
B, S, K = 64, 200, 50
V, E = 100000, 128
KE = K + 1            # items + bias slot
BIAS_V = V            # logical row V   = b
ZERO_V = V + 1        # logical row V+1 = 0
NV = V + 2            # logical vocab incl. bias+zero rows
P = 128
B_PER_CORE = B // N_CORES
BAGS = B_PER_CORE * S               # 1600 bags per core
N_TILES = -(-BAGS // P)             # 13
BAGS_PAD = N_TILES * P              # 1664

CHUNK_CAP = 32767                   # real rows per chunk (idx 1..32767)
N_CHUNKS = -(-NV // CHUNK_CAP)      # 4
CHUNK_STRIDE = 32768                # rows per chunk incl. its zero row
CHUNK_ROWS = [min(CHUNK_CAP, NV - CHUNK_CAP * c) + 1 for c in range(N_CHUNKS)]

TRACE = False       # test.py flips this to profile
LAST_RESULTS = {}   # test.py reads exec_time_ns etc. from here


def _build_tables(W, b):
    """fp16 chunk tables, each [zero row; <=CHUNK_CAP vocab rows].

    Separate tensors (not slices of one): the gather ucode's row addressing
    breaks when AP-base-offset + idx exceeds 32767 rows."""
    wt = np.zeros((NV, E), np.float32)
    wt[:V] = W.T
    wt[BIAS_V] = b
    tabs = []
    for c in range(N_CHUNKS):
        real = CHUNK_ROWS[c] - 1
        t = np.zeros((CHUNK_ROWS[c], E), np.float16)
        t[1:] = wt[CHUNK_CAP * c : CHUNK_CAP * c + real]
        tabs.append(t)
    return tabs


def _plan_core(v_bags):
    """v_bags: [BAGS, KE] logical rows. Returns per-(tile,chunk) raw widths and
    per-bag chunk-sorted idx lists.

    sorted_idx[bag, j] = local int16 idx of the bag's j-th slot when slots are
    ordered by chunk; cnt[bag, c] = number of slots in chunk c."""
    C = v_bags // CHUNK_CAP                      # [BAGS, KE] chunk of each slot
    IDX = (v_bags - C * CHUNK_CAP + 1).astype(np.int16)
    order = np.argsort(C, axis=1, kind="stable")  # chunk-major slot order
    C_sorted = np.take_along_axis(C, order, axis=1)
    IDX_sorted = np.take_along_axis(IDX, order, axis=1)
    cnt = np.stack([(C == c).sum(axis=1) for c in range(N_CHUNKS)], axis=1)
    return C_sorted, IDX_sorted, cnt


def _cluster(cnt):
    """Greedy-pack 1600 bags into 13 tiles of 128 minimizing sum of per-tile
    per-chunk maxima. Returns tiles [N_TILES, P] of bag ids (-1 = dummy)."""
    order = np.argsort(-cnt.max(axis=1), kind="stable")
    m = np.zeros((N_TILES, N_CHUNKS), np.int64)
    fill = np.zeros(N_TILES, np.int64)
    tiles = np.full((N_TILES, P), -1, np.int64)
    for b in order:
        best_key, best_t = None, None
        for t in range(N_TILES):
            if fill[t] >= P:
                continue
            inc = int(np.maximum(m[t], cnt[b]).sum() - m[t].sum())
            key = (inc, -int(fill[t]))
            if best_key is None or key < best_key:
                best_key, best_t = key, t
        tiles[best_t, fill[best_t]] = b
        m[best_t] = np.maximum(m[best_t], cnt[b])
        fill[best_t] += 1
    # sort tiles by descending total width so tiles align across cores
    tw = m.sum(axis=1)
    order_t = np.argsort(-tw, kind="stable")
    return tiles[order_t], m[order_t]


def _wrap_idxs(arr, w):
    """arr [P, w] int16 (partition-major slot grid) -> [128, w*8] wrapped+replicated."""
    L = P * w
    i = np.arange(L)
    lin = arr[i % P, i // P]                     # list position i = col*128 + p
    wrapped = lin.reshape(w * 8, 16).T           # [16, w*8]
    return np.tile(wrapped, (8, 1)).astype(np.int16)


def _prep_inputs(content_input, W, b):
    """Returns (in_maps, widths) where widths[t][c] is shared across cores."""
    ids = np.asarray(content_input).astype(np.int64).reshape(B, S, K)
    Wf = np.asarray(W, dtype=np.float32)
    bf = np.asarray(b, dtype=np.float32)
    tabs = _build_tables(Wf, bf)

    ids = np.where(ids == 0, ZERO_V, ids)
    per_core = []
    tiles_per_core = []
    widths = np.zeros((N_TILES, N_CHUNKS), np.int64)
    for c in range(N_CORES):
        bag_ids = ids[c * B_PER_CORE : (c + 1) * B_PER_CORE].reshape(BAGS, K)
        v = np.concatenate(
            [bag_ids, np.full((BAGS, 1), BIAS_V, np.int64)], axis=1
        )  # [BAGS, KE]
        plan = _plan_core(v)
        per_core.append(plan)
        tiles, m = _cluster(plan[2])
        tiles_per_core.append(tiles)
        widths = np.maximum(widths, m)

    in_maps = []
    for core in range(N_CORES):
        _, IDX_sorted, cnt = per_core[core]
        tiles = tiles_per_core[core]
        planes = []
        for t in range(N_TILES):
            bags = tiles[t]  # [P] bag ids, -1 = dummy
            real = bags >= 0
            bsafe = np.where(real, bags, 0)
            csum = np.zeros((P,), np.int64)
            for ch in range(N_CHUNKS):
                w = int(widths[t][ch])
                if w == 0:
                    continue
                cn = np.where(real, cnt[bsafe, ch], 0)
                j = np.arange(w)[None, :]
                src = np.take_along_axis(
                    IDX_sorted[bsafe], np.minimum(csum[:, None] + j, KE - 1), axis=1
                )
                arr = np.where((j < cn[:, None]) & real[:, None], src, 0).astype(
                    np.int16
                )
                planes.append(_wrap_idxs(arr, w))
                csum += cn
        ids16 = np.concatenate(planes, axis=1)
        m = {"ids16": np.ascontiguousarray(ids16)}
        for c in range(N_CHUNKS):
            m[f"wt{c}"] = tabs[c]
        in_maps.append(m)
    return in_maps, widths, tiles_per_core


def _build_program(widths, ids_cols):
    import concourse.bass as bass
    import concourse.mybir as mybir
    from concourse import bacc
    from concourse.tile import TileContext

    sw = [int(widths[t].sum()) for t in range(N_TILES)]
    sw_max = max(sw)

    nc = bacc.Bacc("TRN2", target_bir_lowering=False, debug=False, num_devices=N_CORES)
    ids_d = nc.declare_dram_parameter("ids16", [P, ids_cols], mybir.dt.int16, isOutput=False)
    wt_ds = [
        nc.declare_dram_parameter(
            f"wt{c}", [CHUNK_ROWS[c], E], mybir.dt.float16, isOutput=False
        )
        for c in range(N_CHUNKS)
    ]
    out_d = nc.declare_dram_parameter("out", [BAGS_PAD, E], mybir.dt.float32, isOutput=True)

    with TileContext(nc) as tc:
        with (
            tc.tile_pool(name="ids", bufs=1) as ids_pool,
            tc.tile_pool(name="g", bufs=3) as g_pool,
            tc.tile_pool(name="o", bufs=3) as o_pool,
        ):
            ids_sb = ids_pool.tile([P, ids_cols], mybir.dt.int16)
            nc.sync.dma_start(out=ids_sb[:], in_=ids_d[:])
            off = 0  # free-dim offset into ids16, in idx elements
            for t in range(N_TILES):
                g = g_pool.tile([P, sw_max * E], mybir.dt.float16)
                col = 0
                for ch in range(N_CHUNKS):
                    w = int(widths[t][ch])
                    if w == 0:
                        continue
                    n = w * P
                    dst = g[:, col * E : (col + w) * E].rearrange(
                        "p (j e) -> p j e", j=w, e=E
                    )
                    nc.gpsimd.dma_gather(
                        dst,
                        wt_ds[ch][:],
                        ids_sb[:, off : off + w * 8],
                        n,
                        n,
                        E,
                        single_packet=False,
                    )
                    col += w
                    off += w * 8
                o = o_pool.tile([P, E], mybir.dt.float32)
                nc.vector.tensor_reduce(
                    out=o[:],
                    in_=g[:, : sw[t] * E].rearrange("p (j e) -> p e j", j=sw[t], e=E),
                    axis=mybir.AxisListType.X,
                    op=mybir.AluOpType.add,
                )
                nc.sync.dma_start(out=out_d[t * P : (t + 1) * P, :], in_=o[:])
    nc.compile()
    return nc


def kernel(content_input, W, b):
    global LAST_RESULTS
    in_maps, widths, tiles_per_core = _prep_inputs(content_input, W, b)
    ids_cols = in_maps[0]["ids16"].shape[1]
    nc = _build_program(widths, ids_cols)

    from concourse.bass_utils import run_bass_kernel_spmd

    res = run_bass_kernel_spmd(nc, in_maps, list(range(N_CORES)), trace=TRACE)
    LAST_RESULTS = {
        "exec_time_ns": res.exec_time_ns,
        "mean_exec_time_ns": res.mean_exec_time_ns,
        "instructions_and_trace": res.instructions_and_trace,
        "profile_json": res.profile_json,
        "widths": widths,
    }

    out = np.empty((B, S, E), np.float32)
    for c in range(N_CORES):
        rows = res.results[c]["out"]  # [BAGS_PAD, E], permuted bag order
        flat = np.empty((BAGS, E), np.float32)
        tiles = tiles_per_core[c].reshape(BAGS_PAD)
        real = tiles >= 0
        flat[tiles[real]] = rows[real]
        out[c * B_PER_CORE : (c + 1) * B_PER_CORE] = flat.reshape(B_PER_CORE, S, E)
    return out


# revision 15
# speedup vs baseline: 1.3306x; 1.0019x over previous
"""Trainium2 Bass kernel for nn_LinearUpscaler (masked embedding-bag sum + bias).

reference:  g = W.T[ids]; g[ids == 0] = 0; out = g.sum(axis=2) + b

Design: data-parallel over batch across 8 cores (8 batch rows -> 1600 bags of
51 slots each: 50 items + 1 bias-row slot; ids==0 remapped to a zero row).

The gather engine is the GPSIMD dma_gather custom op (int16 indices, one
512B/256B row per index, written to partition i%128, column i//128).  Since
indices are signed int16 (max 32767) and V=100002, the fp16 table is split
into 4 vocab chunks; row 0 of each chunk is a zero row so padding slots can
gather harmlessly.  For each tile of 128 bags, each bag's slots are bucketed
by chunk; per (tile, chunk) all bags are padded to a common width W so the
dma_gather index list is fully valid (no negative indices) with a
compile-time num_idxs.  The list order is chosen so bag b's rows land in
partition b at consecutive columns; one strided vector-engine reduce per tile
sums items+chunks+padding (pads contribute zeros) in a single pass, and the
result is DMA'd out.  The program is specialized per call (widths depend on
the actual ids); no collectives are needed.
"""

import importlib.util
import os
import sys

if importlib.util.find_spec("concourse") is None:
    for _p in ("/opt/trn_rl_repo", "/root/.axon_site/_ro/trn_rl_repo"):
        if os.path.isdir(_p) and _p not in sys.path:
            sys.path.insert(0, _p)
            break

import numpy as np

N_CORES = 8
B, S, K = 64, 200, 50
V, E = 100000, 128
KE = K + 1            # items + bias slot
BIAS_V = V            # logical row V   = b
ZERO_V = V + 1        # logical row V+1 = 0
NV = V + 2            # logical vocab incl. bias+zero rows
P = 128
B_PER_CORE = B // N_CORES
BAGS = B_PER_CORE * S               # 1600 bags per core
N_TILES = -(-BAGS // P)             # 13
BAGS_PAD = N_TILES * P              # 1664

CHUNK_CAP = 32767                   # real rows per chunk (idx 1..32767)
N_CHUNKS = -(-NV // CHUNK_CAP)      # 4
CHUNK_STRIDE = 32768                # rows per chunk incl. its zero row
CHUNK_ROWS = [min(CHUNK_CAP, NV - CHUNK_CAP * c) + 1 for c in range(N_CHUNKS)]

TRACE = False       # test.py flips this to profile
LAST_RESULTS = {}   # test.py reads exec_time_ns etc. from here


def _build_tables(W, b):
    """fp16 chunk tables, each [zero row; <=CHUNK_CAP vocab rows].

    Separate tensors (not slices of one): the gather ucode's row addressing
    breaks when AP-base-offset + idx exceeds 32767 rows."""
    wt = np.zeros((NV, E), np.float32)
    wt[:V] = W.T
    wt[BIAS_V] = b
    tabs = []
    for c in range(N_CHUNKS):
        real = CHUNK_ROWS[c] - 1
        t = np.zeros((CHUNK_ROWS[c], E), np.float16)
        t[1:] = wt[CHUNK_CAP * c : CHUNK_CAP * c + real]
        tabs.append(t)
    return tabs


def _plan_core(v_bags):
    """v_bags: [BAGS, KE] logical rows. Returns per-(tile,chunk) raw widths and
    per-bag chunk-sorted idx lists.

    sorted_idx[bag, j] = local int16 idx of the bag's j-th slot when slots are
    ordered by chunk; cnt[bag, c] = number of slots in chunk c."""
    C = v_bags // CHUNK_CAP                      # [BAGS, KE] chunk of each slot
    IDX = (v_bags - C * CHUNK_CAP + 1).astype(np.int16)
    order = np.argsort(C, axis=1, kind="stable")  # chunk-major slot order
    C_sorted = np.take_along_axis(C, order, axis=1)
    IDX_sorted = np.take_along_axis(IDX, order, axis=1)
    cnt = np.stack([(C == c).sum(axis=1) for c in range(N_CHUNKS)], axis=1)
    return C_sorted, IDX_sorted, cnt


def _cluster_once(cnt, order):
    m = np.zeros((N_TILES, N_CHUNKS), np.int64)
    fill = np.zeros(N_TILES, np.int64)
    tiles = np.full((N_TILES, P), -1, np.int64)
    for b in order:
        best_key, best_t = None, None
        for t in range(N_TILES):
            if fill[t] >= P:
                continue
            inc = int(np.maximum(m[t], cnt[b]).sum() - m[t].sum())
            key = (inc, -int(fill[t]))
            if best_key is None or key < best_key:
                best_key, best_t = key, t
        tiles[best_t, fill[best_t]] = b
        m[best_t] = np.maximum(m[best_t], cnt[b])
        fill[best_t] += 1
    return tiles, m


def _cluster(cnt):
    """Greedy-pack 1600 bags into 13 tiles of 128 minimizing sum of per-tile
    per-chunk maxima; best of a few orderings. Returns tiles [N_TILES, P] of
    bag ids (-1 = dummy)."""
    orders = [
        np.argsort(-cnt.max(axis=1), kind="stable"),
        np.argsort(-cnt[:, :3].max(axis=1), kind="stable"),
        np.lexsort((cnt[:, 2], cnt[:, 1], cnt[:, 0]))[::-1],
    ]
    best = None
    for order in orders:
        tiles, m = _cluster_once(cnt, order)
        tot = int(m.sum())
        if best is None or tot < best[0]:
            best = (tot, tiles, m)
    _, tiles, m = best
    # sort tiles by descending total width so tiles align across cores
    tw = m.sum(axis=1)
    order_t = np.argsort(-tw, kind="stable")
    return tiles[order_t], m[order_t]


def _wrap_idxs(arr, w):
    """arr [P, w] int16 (partition-major slot grid) -> [128, w*8] wrapped+replicated."""
    L = P * w
    i = np.arange(L)
    lin = arr[i % P, i // P]                     # list position i = col*128 + p
    wrapped = lin.reshape(w * 8, 16).T           # [16, w*8]
    return np.tile(wrapped, (8, 1)).astype(np.int16)


def _prep_inputs(content_input, W, b):
    """Returns (in_maps, widths) where widths[t][c] is shared across cores."""
    ids = np.asarray(content_input).astype(np.int64).reshape(B, S, K)
    Wf = np.asarray(W, dtype=np.float32)
    bf = np.asarray(b, dtype=np.float32)
    tabs = _build_tables(Wf, bf)

    ids = np.where(ids == 0, ZERO_V, ids)
    per_core = []
    tiles_per_core = []
    widths = np.zeros((N_TILES, N_CHUNKS), np.int64)
    for c in range(N_CORES):
        bag_ids = ids[c * B_PER_CORE : (c + 1) * B_PER_CORE].reshape(BAGS, K)
        v = np.concatenate(
            [bag_ids, np.full((BAGS, 1), BIAS_V, np.int64)], axis=1
        )  # [BAGS, KE]
        plan = _plan_core(v)
        per_core.append(plan)
        tiles, m = _cluster(plan[2])
        tiles_per_core.append(tiles)
        widths = np.maximum(widths, m)

    in_maps = []
    for core in range(N_CORES):
        _, IDX_sorted, cnt = per_core[core]
        tiles = tiles_per_core[core]
        planes = []
        for t in range(N_TILES):
            bags = tiles[t]  # [P] bag ids, -1 = dummy
            real = bags >= 0
            bsafe = np.where(real, bags, 0)
            csum = np.zeros((P,), np.int64)
            for ch in range(N_CHUNKS):
                w = int(widths[t][ch])
                if w == 0:
                    continue
                cn = np.where(real, cnt[bsafe, ch], 0)
                j = np.arange(w)[None, :]
                src = np.take_along_axis(
                    IDX_sorted[bsafe], np.minimum(csum[:, None] + j, KE - 1), axis=1
                )
                arr = np.where((j < cn[:, None]) & real[:, None], src, 0).astype(
                    np.int16
                )
                planes.append(_wrap_idxs(arr, w))
                csum += cn
        ids16 = np.concatenate(planes, axis=1)
        m = {"ids16": np.ascontiguousarray(ids16)}
        for c in range(N_CHUNKS):
            m[f"wt{c}"] = tabs[c]
        in_maps.append(m)
    return in_maps, widths, tiles_per_core


def _build_program(widths, ids_cols):
    import concourse.bass as bass
    import concourse.mybir as mybir
    from concourse import bacc
    from concourse.tile import TileContext

    sw = [int(widths[t].sum()) for t in range(N_TILES)]
    sw_max = max(sw)

    nc = bacc.Bacc("TRN2", target_bir_lowering=False, debug=False, num_devices=N_CORES)
    ids_d = nc.declare_dram_parameter("ids16", [P, ids_cols], mybir.dt.int16, isOutput=False)
    wt_ds = [
        nc.declare_dram_parameter(
            f"wt{c}", [CHUNK_ROWS[c], E], mybir.dt.float16, isOutput=False
        )
        for c in range(N_CHUNKS)
    ]
    out_d = nc.declare_dram_parameter("out", [BAGS_PAD, E], mybir.dt.float32, isOutput=True)

    with TileContext(nc) as tc:
        with (
            tc.tile_pool(name="ids", bufs=1) as ids_pool,
            tc.tile_pool(name="g", bufs=3) as g_pool,
            tc.tile_pool(name="o", bufs=3) as o_pool,
        ):
            ids_sb = ids_pool.tile([P, ids_cols], mybir.dt.int16)
            nc.sync.dma_start(out=ids_sb[:], in_=ids_d[:])
            off = 0  # free-dim offset into ids16, in idx elements
            for t in range(N_TILES):
                g = g_pool.tile([P, sw_max * E], mybir.dt.float16)
                col = 0
                for ch in range(N_CHUNKS):
                    w = int(widths[t][ch])
                    if w == 0:
                        continue
                    n = w * P
                    dst = g[:, col * E : (col + w) * E].rearrange(
                        "p (j e) -> p j e", j=w, e=E
                    )
                    nc.gpsimd.dma_gather(
                        dst,
                        wt_ds[ch][:],
                        ids_sb[:, off : off + w * 8],
                        n,
                        n,
                        E,
                        single_packet=False,
                    )
                    col += w
                    off += w * 8
                o = o_pool.tile([P, E], mybir.dt.float32)
                nc.vector.tensor_reduce(
                    out=o[:],
                    in_=g[:, : sw[t] * E].rearrange("p (j e) -> p e j", j=sw[t], e=E),
                    axis=mybir.AxisListType.X,
                    op=mybir.AluOpType.add,
                )
                nc.sync.dma_start(out=out_d[t * P : (t + 1) * P, :], in_=o[:])
    nc.compile()
    return nc


def kernel(content_input, W, b):
    global LAST_RESULTS
    in_maps, widths, tiles_per_core = _prep_inputs(content_input, W, b)
    ids_cols = in_maps[0]["ids16"].shape[1]
    nc = _build_program(widths, ids_cols)

    from concourse.bass_utils import run_bass_kernel_spmd

    res = run_bass_kernel_spmd(nc, in_maps, list(range(N_CORES)), trace=TRACE)
    LAST_RESULTS = {
        "exec_time_ns": res.exec_time_ns,
        "mean_exec_time_ns": res.mean_exec_time_ns,
        "instructions_and_trace": res.instructions_and_trace,
        "profile_json": res.profile_json,
        "widths": widths,
    }

    out = np.empty((B, S, E), np.float32)
    for c in range(N_CORES):
        rows = res.results[c]["out"]  # [BAGS_PAD, E], permuted bag order
        flat = np.empty((BAGS, E), np.float32)
        tiles = tiles_per_core[c].reshape(BAGS_PAD)
        real = tiles >= 0
        flat[tiles[real]] = rows[real]
        out[c * B_PER_CORE : (c + 1) * B_PER_CORE] = flat.reshape(B_PER_CORE, S, E)
    return out
